# revision 1
# baseline (speedup 1.0000x reference)
"""ALiBi bias add on 8 Trainium2 NeuronCores.

out[b, h, i, j] = attention_scores[b, h, i, j] + slopes[h] * (j - i)

Fully elementwise and memory-bound: 512 MB read + 512 MB write per chip.

Sharding: the 32 (batch, head) slices are split as 2 heads x 2 batches per
core (core c owns heads {2c, 2c+1} for both batches), so each core streams
4 x [2048, 2048] slices through SBUF.

Bias trick: the ALiBi bias is Toeplitz. For the 128-row tile starting at
row r0 = 128k, bias[p, j] = slope * ((j - 128k) - p), which is a
column-shifted window of one extended table
    ebase[p, x] = slope * (x - 1920 - p),  x in [0, 3968)
kept in SBUF (one 2 MB table per head). Every [128, 2048] tile then needs
exactly one DVE tensor_add against ebase[:, 1920-128k : 3968-128k] -- no
per-tile bias generation and no extra HBM traffic beyond the 2 tables.
The tables are precomputed on host from the tiny slopes vector (f32 ops
identical to the reference's, so the result is bit-exact).
"""

import numpy as np

B, H, S = 2, 16, 2048
P = 128                # SBUF partitions
NT = S // P            # 16 row tiles per slice
W = S + (NT - 1) * P   # 3968: extended bias table width
N_CORES = 8
HPC = H // N_CORES     # 2 heads per core
SLICES = B * HPC       # 4 (batch, head) slices per core

_built = None


def _build():
    """Build + compile the per-core Bass graph (cached)."""
    global _built
    if _built is not None:
        return _built

    import concourse.tile as tile
    from concourse import bacc, mybir

    nc = bacc.Bacc("TRN2", target_bir_lowering=False, debug=False,
                   num_devices=N_CORES)
    scores = nc.dram_tensor("scores", [SLICES, S, S], mybir.dt.float32,
                            kind="ExternalInput").ap()
    ebase = nc.dram_tensor("ebase", [HPC, P, W], mybir.dt.float32,
                           kind="ExternalInput").ap()
    out = nc.dram_tensor("out", [SLICES, S, S], mybir.dt.float32,
                         kind="ExternalOutput").ap()

    with tile.TileContext(nc) as tc:
        with tc.tile_pool(name="const", bufs=1) as cpool, \
             tc.tile_pool(name="work", bufs=8) as pool:
            eb = cpool.tile([P, HPC * W], mybir.dt.float32)
            for t in range(HPC):
                nc.sync.dma_start(eb[:, t * W:(t + 1) * W], ebase[t])
            for s in range(SLICES):
                t = s % HPC  # head slot within this core
                for k in range(NT):
                    tl = pool.tile([P, S], mybir.dt.float32)
                    nc.sync.dma_start(tl[:], scores[s, k * P:(k + 1) * P, :])
                    off = t * W + (NT - 1 - k) * P
                    nc.vector.tensor_add(tl[:], tl[:], eb[:, off:off + S])
                    nc.scalar.dma_start(out[s, k * P:(k + 1) * P, :], tl[:])
    nc.compile()
    _built = nc
    return _built


def _shard(scores, slopes):
    """Full [B,H,S,S] scores + [H] slopes -> per-core in_maps."""
    xs = (np.arange(W, dtype=np.float32)[None, :]
          - np.float32(S - P)
          - np.arange(P, dtype=np.float32)[:, None])  # [128, W]: x - 1920 - p
    in_maps = []
    for c in range(N_CORES):
        hs = range(HPC * c, HPC * (c + 1))
        sl = np.stack([scores[b, h] for b in range(B) for h in hs])
        eb = np.stack([np.float32(slopes[h]) * xs for h in hs])
        in_maps.append({"scores": sl, "ebase": eb})
    return in_maps


def _unshard(results):
    out = np.empty((B, H, S, S), np.float32)
    for c in range(N_CORES):
        r = results[c]["out"]
        for b in range(B):
            for t in range(HPC):
                out[b, HPC * c + t] = r[b * HPC + t]
    return out


def run(attention_scores, slopes, **spmd_kwargs):
    """Shard, execute on 8 cores, gather. Returns (output, BassKernelResults)."""
    from concourse.bass_utils import run_bass_kernel_spmd

    nc = _build()
    scores = np.ascontiguousarray(attention_scores, dtype=np.float32)
    slopes = np.asarray(slopes, dtype=np.float32)
    in_maps = _shard(scores, slopes)
    res = run_bass_kernel_spmd(nc, in_maps, core_ids=list(range(N_CORES)),
                               **spmd_kwargs)
    return _unshard(res.results), res


def kernel(attention_scores, slopes, seq_len=None, **_unused):
    out, _ = run(attention_scores, slopes)
    return out
